# revision 10
# baseline (speedup 1.0000x reference)
"""Causal multi-head self-attention with RoPE on 8 Trainium2 NeuronCores.

Sharding: batch (4) x head-half (2) -> 8 self-contained cores. Each core
computes Q/K/V projections for its 8 heads, RoPE, causal flash-style
attention (scores kept transposed [key, query] so probs feed the V matmul
with no on-device transpose), and a partial output projection over its 512
context features. The two partial outputs per batch are summed on host
(the "all-reduce after output projection" of the tensor-parallel split).

Device layout notes:
- All matmuls run as float32r (full fp32 data, fast PE path).
- Softmax: scores are masked additively via a PE-written -1e30 triangle,
  exponentiated without max-subtraction (scores are bounded; verified),
  and the per-query sums come free from a ones-column appended to V.
- RoPE is evaluated as q*cos + swap(q)*(+-sin) where swap is a DVE
  stream_shuffle partition pair-swap and the sign is folded into the sin
  table.
"""

import sys

sys.path.insert(0, "/opt/trn_rl_repo")

import numpy as np

B, S_FULL, D, H = 4, 2048, 1024, 16
DK = 64  # head dim
HL = 8  # heads per core
DL = HL * DK  # 512 local features
ROPE_THETA = 10000.0
NEG = -1.0e30

_CACHE = {}


def _emit(nc, tc, tensors, S, reps=1):
    import concourse.tile as tile  # noqa: F401
    from concourse import mybir
    from contextlib import ExitStack

    f32, f32r = mybir.dt.float32, mybir.dt.float32r
    AF = mybir.ActivationFunctionType
    SWAP = [i ^ 1 for i in range(32)]
    NSB = S // 512  # query super-blocks
    NKB = S // 128  # key blocks
    DEPTH = 2  # kj-iterations the V-matmul trails the score matmul by

    xT, wqT, wkT, wvT, woT = (
        tensors["xT"], tensors["wqT"], tensors["wkT"], tensors["wvT"], tensors["woT"],
    )
    c2, s2m, maskT, ident = (
        tensors["c2"], tensors["s2m"], tensors["maskT"], tensors["ident"],
    )
    ones65, vones, outp = tensors["ones65"], tensors["vones"], tensors["outp"]

    with ExitStack() as ctx:
        const = ctx.enter_context(tc.tile_pool(name="const", bufs=1))
        wres = ctx.enter_context(tc.tile_pool(name="wres", bufs=1))
        xt_p = ctx.enter_context(tc.tile_pool(name="xt", bufs=8))
        tb_p = ctx.enter_context(tc.tile_pool(name="tb", bufs=2))
        kt_p = ctx.enter_context(tc.tile_pool(name="kt", bufs=1))
        vt_p = ctx.enter_context(tc.tile_pool(name="vt", bufs=1))
        qt_p = ctx.enter_context(tc.tile_pool(name="qt", bufs=1))
        qs_p = ctx.enter_context(tc.tile_pool(name="qs", bufs=1))
        ex_p = ctx.enter_context(tc.tile_pool(name="ex", bufs=DEPTH + 1))
        cx_p = ctx.enter_context(tc.tile_pool(name="cx", bufs=1))
        rc_p = ctx.enter_context(tc.tile_pool(name="rc", bufs=1))
        rl_p = ctx.enter_context(tc.tile_pool(name="rl", bufs=1))
        os_p = ctx.enter_context(tc.tile_pool(name="os", bufs=1))
        pp = ctx.enter_context(tc.tile_pool(name="pp", bufs=2, space="PSUM"))
        ps = ctx.enter_context(tc.tile_pool(name="ps", bufs=2, space="PSUM"))
        pc = ctx.enter_context(tc.tile_pool(name="pc", bufs=1, space="PSUM"))

        # small constants (SP queue)
        maskT_t = const.tile([128, 896], f32r, tag="maskT")
        nc.sync.dma_start(maskT_t[:], maskT[:].bitcast(f32r))
        ident_t = const.tile([128, 128], f32r, tag="ident")
        nc.sync.dma_start(ident_t[:], ident[:].bitcast(f32r))
        ones_t = const.tile([65, 64], f32r, tag="ones")
        nc.sync.dma_start(ones_t[:], ones65[:].bitcast(f32r))

        # resident weights (contraction-major), loaded per 128-row chunk (Pool queue)
        woT_r = woT.rearrange("(t p) o -> p t o", p=128)
        # prefetch x tiles and rope tables for the first super-block before
        # the big weight loads so the first projection starts early
        xts0 = []
        for ct in range(8):
            xt_t = xt_p.tile([128, 512], f32r, tag="xt", name="xt0")
            nc.gpsimd.dma_start(xt_t[:], xT[ct * 128 : (ct + 1) * 128, 0:512].bitcast(f32r))
            xts0.append(xt_t)
        c2_t0 = tb_p.tile([128, 512], f32, tag="c2s", name="c2s0")
        nc.sync.dma_start(c2_t0[:], c2[:, 0:512])
        s2m_t0 = tb_p.tile([128, 512], f32, tag="s2s", name="s2s0")
        nc.sync.dma_start(s2m_t0[:], s2m[:, 0:512])
        wq_t = wres.tile([128, 8, DL], f32r, tag="wq")
        wk_t = wres.tile([128, 8, DL], f32r, tag="wk")
        wv_t = wres.tile([128, 8, DL], f32r, tag="wv")
        wo_t = wres.tile([128, 4, D], f32r, tag="wo")
        for w_t, wT in ((wk_t, wkT), (wq_t, wqT), (wv_t, wvT)):
            for ct in range(8):
                nc.gpsimd.dma_start(
                    w_t[:, ct, :], wT[ct * 128 : (ct + 1) * 128, :].bitcast(f32r)
                )
        for it in range(4):
            nc.sync.dma_start(
                wo_t[:, it, :], woT_r[:, it, :].bitcast(f32r)
            )

        # persistent K / V buffers
        kt_tiles = {}
        for hp in range(4):
            for sbk in range(NSB):
                kt_tiles[hp, sbk] = kt_p.tile([128, 512], f32r, tag=f"kt{hp}_{sbk}", name=f"kt{hp}_{sbk}")
        v_tiles = {}
        for kb in range(NKB):
            v_tiles[kb] = vt_p.tile([128, HL, 65], f32r, tag=f"v{kb}", name=f"v{kb}")
            nc.sync.dma_start(
                v_tiles[kb][:, :, 64:65],
                vones.rearrange("p (h o) -> p h o", o=1).bitcast(f32r),
            )

        for _rep in range(reps):
         for sb in range(NSB):
            s0 = sb * 512
            if _rep == 0 and sb == 0:
                xts, c2_t, s2m_t = xts0, c2_t0, s2m_t0
            else:
                xts = []
                for ct in range(8):
                    xt_t = xt_p.tile([128, 512], f32r, tag="xt")
                    nc.gpsimd.dma_start(
                        xt_t[:], xT[ct * 128 : (ct + 1) * 128, s0 : s0 + 512].bitcast(f32r)
                    )
                    xts.append(xt_t)
                c2_t = tb_p.tile([128, 512], f32, tag="c2s")
                nc.sync.dma_start(c2_t[:], c2[:, s0 : s0 + 512])
                s2m_t = tb_p.tile([128, 512], f32, tag="s2s")
                nc.sync.dma_start(s2m_t[:], s2m[:, s0 : s0 + 512])

            # K and Q projections (transposed layout [f, s]) + RoPE
            qt_tiles = {}
            for w_t, is_k in ((wk_t, True), (wq_t, False)):
                for ft in range(4):
                    pr = pp.tile([128, 512], f32, tag="mm")
                    for ct in range(8):
                        nc.tensor.matmul(
                            pr[:],
                            w_t[:, ct, ft * 128 : (ft + 1) * 128],
                            xts[ct][:],
                            start=(ct == 0),
                            stop=(ct == 7),
                        )
                    # rope: dst = pr * cos + pairswap(pr) * (+-sin)
                    qs_t = qs_p.tile([128, 512], f32, tag="qs")
                    nc.vector.stream_shuffle(qs_t[:], pr[:], SWAP)
                    nc.gpsimd.tensor_mul(qs_t[:], qs_t[:], s2m_t[:])
                    if is_k:
                        dst = kt_tiles[ft, sb]
                    else:
                        dst = qt_p.tile([128, 512], f32r, tag=f"qt{ft}", name=f"qt{ft}")
                        qt_tiles[ft] = dst
                    nc.vector.tensor_mul(dst[:], pr[:], c2_t[:])
                    nc.vector.tensor_add(dst[:], dst[:], qs_t[:])

            # V projection (natural layout [s, f]) into the ones-augmented tiles
            for i in range(4):
                kb = sb * 4 + i
                pr = pp.tile([128, 512], f32, tag="mm")
                for ct in range(8):
                    nc.tensor.matmul(
                        pr[:],
                        xts[ct][:, i * 128 : (i + 1) * 128],
                        wv_t[:, ct, :],
                        start=(ct == 0),
                        stop=(ct == 7),
                    )
                nc.vector.tensor_copy(
                    v_tiles[kb][:, :, 0:64], pr[:].rearrange("p (h d) -> p h d", h=HL)
                )

            # attention for this query super-block, score->exp->V software-pipelined
            n_kj = 4 * (sb + 1)
            cx_tiles = {}
            for hp in range(4):
                cxs = (
                    pc.tile([65, 512], f32, tag="ce", name="ce"),
                    pc.tile([65, 512], f32, tag="co", name="co"),
                )
                exts = {}
                spans = {}

                def emit_v(kj):
                    qo, w = spans[kj]
                    for par in (0, 1):
                        nc.tensor.matmul(
                            cxs[par][:, qo : qo + w],
                            v_tiles[kj][:, hp * 2 + par, :],
                            exts[kj, par][:, 0:w],
                            start=(kj == 0),
                            stop=(kj == n_kj - 1),
                        )
                    del exts[kj, 0], exts[kj, 1]

                for kj in range(n_kj):
                    diag = kj >= 4 * sb
                    kjl = kj - 4 * sb
                    w = max(512 - 128 * kjl, 256) if diag else 512
                    qo = 512 - w
                    moff = 384 - (128 * kjl - qo)
                    sbk, col = kj // 4, (kj % 4) * 128
                    spans[kj] = (qo, w)
                    for par in (0, 1):
                        bp = 64 * par
                        kt_sl = kt_tiles[hp, sbk][bp : bp + 64, col : col + 128]
                        qt_sl = qt_tiles[hp][bp : bp + 64, qo : qo + w]
                        scp = ps.tile([128, 512], f32, tag=f"sc{par}", name=f"sc{par}")
                        if diag:
                            nc.tensor.matmul(
                                scp[:, qo : qo + w],
                                ident_t[:],
                                maskT_t[:, moff : moff + w],
                                start=True,
                                stop=False,
                            )
                            nc.tensor.matmul(
                                scp[:, qo : qo + w], kt_sl, qt_sl, start=False, stop=True
                            )
                        else:
                            nc.tensor.matmul(scp[:], kt_sl, qt_sl, start=True, stop=True)
                        ext = ex_p.tile([128, 512], f32r, tag=f"ex{par}", name=f"ex{par}")
                        nc.scalar.activation(
                            ext[:, 0:w], scp[:, qo : qo + w], AF.Exp, scale=0.125
                        )
                        exts[kj, par] = ext
                    if kj >= DEPTH:
                        emit_v(kj - DEPTH)
                for kj in range(max(0, n_kj - DEPTH), n_kj):
                    emit_v(kj)

                # normalize: ctx rows 0..63, sum(exp) in row 64
                cxt = cx_p.tile([128, 512], f32r, tag=f"cx{hp}", name=f"cx{hp}")
                cx_tiles[hp] = cxt
                for par in (0, 1):
                    rc_t = rc_p.tile([65, 512], f32r, tag="rc")
                    with nc.allow_low_precision(reason="softmax reciprocal to f32r"):
                        nc.vector.reciprocal(rc_t[64:65, :], cxs[par][64:65, :])
                    rbp = pp.tile([64, 512], f32, tag="mm")
                    nc.tensor.matmul(
                        rbp[:], ones_t[64:65, :], rc_t[64:65, :], start=True, stop=True
                    )
                    rbs = rc_p.tile([64, 512], f32, tag="rbs")
                    nc.scalar.copy(rbs[:], rbp[:])
                    if par == 0:
                        nc.vector.tensor_mul(cxt[0:64, :], cxs[par][0:64, :], rbs[:])
                    else:
                        rl_t = rl_p.tile([64, 512], f32r, tag="rl")
                        nc.vector.tensor_mul(rl_t[:], cxs[par][0:64, :], rbs[:])
                        nc.sync.dma_start(cxt[64:128, :], rl_t[:])

            # output projection for this super-block (partial over 512 features)
            for ob in range(2):
                for sq in range(4):
                    opp = pp.tile([128, 512], f32, tag="mm")
                    for hp in range(4):
                        nc.tensor.matmul(
                            opp[:],
                            cx_tiles[hp][:, sq * 128 : (sq + 1) * 128],
                            wo_t[:, hp, ob * 512 : (ob + 1) * 512],
                            start=(hp == 0),
                            stop=(hp == 3),
                        )
                    ost = os_p.tile([128, 512], f32, tag="os")
                    nc.scalar.copy(ost[:], opp[:])
                    nc.sync.dma_start(
                        outp[s0 + sq * 128 : s0 + (sq + 1) * 128, ob * 512 : (ob + 1) * 512],
                        ost[:],
                    )


def build(S=S_FULL, reps=1, chain=False):
    import concourse.tile as tile
    from concourse import bacc, mybir

    f32 = mybir.dt.float32
    nc = bacc.Bacc(None, target_bir_lowering=False, debug=False)
    t = {}
    t["xT"] = nc.dram_tensor("xT", [D, S], f32, kind="ExternalInput")
    t["wqT"] = nc.dram_tensor("wqT", [D, DL], f32, kind="ExternalInput")
    t["wkT"] = nc.dram_tensor("wkT", [D, DL], f32, kind="ExternalInput")
    t["wvT"] = nc.dram_tensor("wvT", [D, DL], f32, kind="ExternalInput")
    t["woT"] = nc.dram_tensor("woT", [DL, D], f32, kind="ExternalInput")
    t["c2"] = nc.dram_tensor("c2", [128, S], f32, kind="ExternalInput")
    t["s2m"] = nc.dram_tensor("s2m", [128, S], f32, kind="ExternalInput")
    t["maskT"] = nc.dram_tensor("maskT", [128, 896], f32, kind="ExternalInput")
    t["ident"] = nc.dram_tensor("ident", [128, 128], f32, kind="ExternalInput")
    t["ones65"] = nc.dram_tensor("ones65", [65, 64], f32, kind="ExternalInput")
    t["vones"] = nc.dram_tensor("vones", [128, HL], f32, kind="ExternalInput")
    t["outp"] = nc.dram_tensor("outp", [S, D], f32, kind="ExternalOutput")
    if chain:
        t["chain"] = nc.dram_tensor("chain", [128, 128], f32, kind="ExternalInput")
        t["chain_out"] = nc.dram_tensor("chain_out", [128, 128], f32, kind="ExternalOutput")

    with tile.TileContext(nc) as tc:
        _emit(nc, tc, t, S, reps=reps)
        if chain:
            with tc.tile_pool(name="chp", bufs=1) as chp:
                cht = chp.tile([128, 128], mybir.dt.float32, name="cht")
                nc.sync.dma_start(cht[:], t["chain"][:])
                nc.sync.dma_start(t["chain_out"][:], cht[:])
    nc.compile()
    return nc


def prep_inputs(x, Wq, Wk, Wv, Wo, token_positions, S=S_FULL):
    x = np.asarray(x)
    Wq, Wk, Wv, Wo = (np.asarray(a) for a in (Wq, Wk, Wv, Wo))
    pos = np.asarray(token_positions).astype(np.float64)
    inv = ROPE_THETA ** (-np.arange(0, DK, 2, dtype=np.float64) / DK)  # [32]
    ang = pos[:, None] * inv[None, :]  # [S, 32]
    cos = np.cos(ang).astype(np.float32).T  # [32, S]
    sin = np.sin(ang).astype(np.float32).T
    i_of_p = (np.arange(128) % 64) // 2
    c2 = np.ascontiguousarray(cos[i_of_p, :])  # [128, S]
    sgn = np.where(np.arange(128) % 2 == 0, -1.0, 1.0).astype(np.float32)
    s2m = np.ascontiguousarray(sin[i_of_p, :] * sgn[:, None])
    maskT = np.where(
        np.arange(896)[None, :] >= np.arange(128)[:, None] + 384, 0.0, NEG
    ).astype(np.float32)
    ident = np.eye(128, dtype=np.float32)
    ones65 = np.ones((65, 64), np.float32)
    vones = np.ones((128, HL), np.float32)

    nb = x.shape[0]
    maps = []
    for c in range(2 * nb):
        b, half = c // 2, c % 2
        rows = slice(half * DL, (half + 1) * DL)
        maps.append(
            {
                "xT": np.ascontiguousarray(x[b].T),
                "wqT": np.ascontiguousarray(Wq[rows].T),
                "wkT": np.ascontiguousarray(Wk[rows].T),
                "wvT": np.ascontiguousarray(Wv[rows].T),
                "woT": np.ascontiguousarray(Wo[:, rows].T),
                "c2": c2,
                "s2m": s2m,
                "maskT": maskT,
                "ident": ident,
                "ones65": ones65,
                "vones": vones,
            }
        )
    return maps


def kernel(x, Wq, Wk, Wv, Wo, token_positions):
    from concourse.bass_utils import run_bass_kernel_spmd

    if "nc" not in _CACHE:
        _CACHE["nc"] = build()
    maps = prep_inputs(x, Wq, Wk, Wv, Wo, token_positions)
    res = run_bass_kernel_spmd(_CACHE["nc"], maps, list(range(8)))
    out = np.empty((B, S_FULL, D), np.float32)
    for b in range(B):
        out[b] = res.results[2 * b]["outp"] + res.results[2 * b + 1]["outp"]
    return out
